# revision 30
# baseline (speedup 1.0000x reference)
"""AttentionPooling (segment softmax-pool) Trainium2 Bass kernel.

Full-input contract: kernel(**inputs) takes the unsharded inputs and
returns the full [1024, 256] float32 output. Internally shards 1024
graphs across 8 NeuronCores (128 contiguous graphs each, node ranges
padded to a common length) and runs one SPMD Bass/Tile kernel.

Math per core (one pass over x):
  h   = tanh((x16 @ W1x8)/128 + b1)  # PE fp8 DoubleRow + ACT
  s   = h @ W2                        # PE, N=1 matmuls -> scores as columns
  e   = exp(s + b2)                   # ACT
  scat[i, 32t+j] = (bl[i,t]==j)*e[i]  # DVE: 2 broadcast tensor_tensor ops
  acc[W(t):W(t)+32, :] += scat_t.T @ x_aug   # PE bf16 x fp8e3, PSUM windows
  out[seg] = acc[seg, 0:256] / (acc[seg, 256] + 1e-8)

The scatter path exploits that `batch` is host-known at Bass build time:
each 128-node tile of the sorted node stream spans at most 2 segments
(every graph in this input has >=128 nodes, host-verified), so each
tile's segments fit a 32-aligned window W(t) of the 128 local segments
(4 windows per core). The per-node window-relative segment id is shipped
in blh, and the scat matrix for a whole 1024-node supergroup is built in
TWO DVE tensor_tensor ops ([P,256]: is_equal against a broadcast iota,
then multiply by broadcast e8) instead of 8 per-tile [P,64] tensor_scalar
ops - DVE drops from pipeline pacer (~1.8us/supergroup) to ~0.6us.
Tiles straddling a window boundary (about 2 per core) get a second
baked matmul+mask targeting the next window. PSUM accumulator is one
[128, 258] f32 bank; the first matmul touching each window uses
start=True (every window matmul writes all 32 rows, zeros included, so
first-touch reset replaces a memset).

Skipping the segment-max subtraction is numerically safe here: |s| is
bounded by ||W2||_1 + |b2| (~7), so exp never overflows fp32.

The score path runs in fp8e4 DoubleRow as before; the scatter path
ships x in fp8e3 scaled by 2.75 (exactly representable; cancels in the
ratio). Data is pre-swizzled on the host so every steady-state DMA is a
contiguous per-partition block per 1024-node supergroup: xct on the
sync ring, xca alternating between the sync and scalar rings (the only
two HWDGE-capable queues on TRN2; issuing from gpsimd falls back to
software descriptor generation and throttles the stream).
"""

import os
from contextlib import ExitStack

import ml_dtypes
import numpy as np

N_CORES = 8
NUM_GRAPHS = 1024
BL = NUM_GRAPHS // N_CORES  # local segments per core = 128
HIDDEN = 256
HH = 128  # mlp hidden
P = 128
GROUP = 512  # nodes per compute group (4 tiles of 128)
SUPER = 1024  # nodes per DMA supergroup (2 compute groups)
XW = 258  # x row width in xa block: 256 features + ones col + pad col
SBL = 64  # segments per half (2 halves per core for padding)
WIN = 32  # psum window width (32-aligned, hardware requirement)
XA_BLK = 4 * XW  # 1032 elems per compute group per partition

XSCALE = 16.0  # x pre-scale before fp8e4 cast for the score path
WSCALE = 8.0  # W1 pre-scale before fp8e4 cast
ACT_SCALE = 1.0 / (XSCALE * WSCALE)
XA_SCALE = 2.75  # x pre-scale before fp8e3 cast for the scatter path; the
# same factor multiplies the ones column, so it cancels in the final
# numerator/denominator ratio. 2.75 is EXACTLY representable in e3m4
# (no systematic ratio bias) and 2.75*max|x| = 14.9 < 15.5 (e3m4 max).


def _build_bass(npad: int, tile_plan, n_extra: int):
    """tile_plan: per tile t a list of (window, blh_col, is_first, is_last)
    matmul entries. blh_col < T are the primary per-supergroup columns;
    cols >= T are straddle extras appended at the end of blh."""
    import concourse.bacc as bacc
    import concourse.mybir as mybir
    import concourse.tile as tile

    dt = mybir.dt
    G = npad // GROUP
    Gd = npad // SUPER
    T = npad // P

    nc = bacc.Bacc("TRN2", target_bir_lowering=False, debug=False)

    xct = nc.dram_tensor("xct", [Gd, P, 2, SUPER], dt.float8e4, kind="ExternalInput")
    xca = nc.dram_tensor("xca", [Gd, P, 2 * XA_BLK], dt.float8e3, kind="ExternalInput")
    blh = nc.dram_tensor("blh", [P, T + n_extra], dt.float32, kind="ExternalInput")
    w1 = nc.dram_tensor("w1", [2, P, HH], dt.float8e4, kind="ExternalInput")
    w2 = nc.dram_tensor("w2", [HH, 1], dt.bfloat16, kind="ExternalInput")
    b1 = nc.dram_tensor("b1", [HH, 1], dt.float32, kind="ExternalInput")
    b2c = nc.dram_tensor("b2c", [P, 1], dt.float32, kind="ExternalInput")
    iotad = nc.dram_tensor("iotad", [P, WIN], dt.float32, kind="ExternalInput")
    out = nc.dram_tensor("out", [BL, HIDDEN], dt.float32, kind="ExternalOutput")

    with tile.TileContext(nc) as tc, ExitStack() as ctx:
        const = ctx.enter_context(tc.tile_pool(name="const", bufs=1))
        xt_pool = ctx.enter_context(tc.tile_pool(name="xt", bufs=8))
        xa_pool = ctx.enter_context(tc.tile_pool(name="xa", bufs=9))
        th_pool = ctx.enter_context(tc.tile_pool(name="th", bufs=3))
        e_pool = ctx.enter_context(tc.tile_pool(name="e", bufs=3))
        sel_pool = ctx.enter_context(tc.tile_pool(name="sel", bufs=2))
        scat_pool = ctx.enter_context(tc.tile_pool(name="scat", bufs=4))
        ex_pool = ctx.enter_context(tc.tile_pool(name="ex", bufs=8))
        fin_pool = ctx.enter_context(tc.tile_pool(name="fin", bufs=1))
        ph_pool = ctx.enter_context(tc.tile_pool(name="ph", bufs=2, space="PSUM"))
        ps_pool = ctx.enter_context(tc.tile_pool(name="ps", bufs=2, space="PSUM"))
        acc_pool = ctx.enter_context(tc.tile_pool(name="acc", bufs=1, space="PSUM"))

        # issue order at boot clock (~800ns/issue): w1 first (gates the PE
        # warm-up), then the first xct halves (gate the first real MLP),
        # then the remaining consts — not needed until tanh/s/exp of
        # iteration 0 (~15µs in). Work started before the wall-anchored HAM
        # ramp still runs at half clock, so earlier data = net gain.
        w1_sb = const.tile([P, 2, HH], dt.float8e4)
        nc.sync.dma_start(w1_sb[:], w1[:].rearrange("h k m -> k h m"))

        # first supergroup of the transposed stream, split in two halves
        # (by node) so the MLP can start before the full supergroup lands
        first_xt_c = xt_pool.tile([P, 2, GROUP], dt.float8e4, tag="xtc0", bufs=1)
        nc.sync.dma_start(first_xt_c[:], xct[0][:, :, 0:GROUP])
        first_xt_r = xt_pool.tile([P, 2, GROUP], dt.float8e4, tag="xtr0", bufs=1)
        nc.sync.dma_start(first_xt_r[:], xct[0][:, :, GROUP : 2 * GROUP])

        b1_sb = const.tile([HH, 1], dt.float32)
        nc.sync.dma_start(b1_sb[:], b1[:])
        w2_sb = const.tile([HH, 1], dt.bfloat16)
        nc.sync.dma_start(w2_sb[:], w2[:])
        b2_sb = const.tile([P, 1], dt.float32)
        nc.sync.dma_start(b2_sb[:], b2c[:])
        iota_sb = const.tile([P, WIN], dt.float32)
        nc.sync.dma_start(iota_sb[:], iotad[:])
        bl_sb = const.tile([P, T + n_extra], dt.float32)
        nc.sync.dma_start(bl_sb[:], blh[:])

        acc = acc_pool.tile([P, XW], dt.float32)

        # out[seg] = acc[seg, 0:256] / (acc[seg, 256] + 1e-8), one PSUM
        # window at a time as soon as its last scatter matmul has landed -
        # 3 of the 4 windows finish mid-kernel, hiding most of the
        # finalize + output-DMA latency from the drain tail.
        sw1 = fin_pool.tile([P, 1], dt.float32, tag="sw")
        recip = fin_pool.tile([P, 1], dt.float32, tag="rc")
        outf = fin_pool.tile([P, HIDDEN], dt.float32, tag="of")

        def finalize_w(w):
            r0, r1 = WIN * w, WIN * (w + 1)
            nc.vector.tensor_scalar_add(
                sw1[r0:r1, :], acc[r0:r1, HIDDEN : HIDDEN + 1], 1e-8
            )
            nc.vector.reciprocal(recip[r0:r1, :], sw1[r0:r1, :])
            nc.vector.tensor_scalar_mul(
                outf[r0:r1, :], acc[r0:r1, 0:HIDDEN], recip[r0:r1, 0:1]
            )
            nc.sync.dma_start(out[r0:r1, :], outf[r0:r1, :])

        # window -> first loop iteration at which its accumulation is done
        fin_iter = {}
        for t in range(len(tile_plan)):
            for (w_, _c, _f, last_) in tile_plan[t]:
                if last_:
                    fin_iter[w_] = t // 8 + 4 + 1
        fin_sched = {}
        for w_, it in fin_iter.items():
            fin_sched.setdefault(it, []).append(w_)

        # PE warm-up: dummy DoubleRow matmuls on the (tiny, early-arriving)
        # weight consts fill the initial xc-DMA wait and bring HAM to K=8/8
        # before the first real matmul. Gating on w1 (rather than a memset
        # tile) makes the warm-up end right as real data lands, so the HAM
        # clock never sees an activity gap and steps back down.
        for _ in range(24):
            wp = ph_pool.tile([HH, 2 * GROUP], dt.float32, tag="psum_h")
            nc.tensor.matmul(
                wp[:, 0:HH], lhsT=w1_sb[:], rhs=w1_sb[:],
                start=True, stop=True,
                perf_mode=mybir.MatmulPerfMode.DoubleRow,
            )

        xtts = {}
        xats = {}

        def dma_load_xt(d):
            t = xt_pool.tile([P, 2, SUPER], dt.float8e4)
            nc.sync.dma_start(t[:], xct[d])
            xtts[d] = t

        def dma_load_xa(d):
            t = xa_pool.tile([P, 2 * XA_BLK], dt.float8e3)
            # prologue burst rides the otherwise-idle scalar queue; in
            # steady state alternate so neither HWDGE queue carries both
            # descriptor-gen and its critical-path work.
            eng = nc.scalar if (d < PREFETCH or d % 2) else nc.sync
            eng.dma_start(t[:], xca[d])
            xats[d] = t

        def xa_slice(g, s):
            t = xats[g // 2]
            base = (g % 2) * XA_BLK + s * XW
            return t[:, base : base + XW]

        def xt_slice(d, n0, n1):
            # [P, 2, n1-n0] fp8 view covering supergroup-local nodes n0:n1
            if d == 0:
                if n1 <= GROUP:
                    return first_xt_c[:, :, n0:n1]
                return first_xt_r[:, :, n0 - GROUP : n1 - GROUP]
            return xtts[d][:, :, n0:n1]

        ths = {}
        phs = {}
        scats = {}   # d -> ([P, 8*WIN] primary scat tile, [(t, extra scat tile)...])

        # The full input stream stays resident in SBUF (~136KB/partition
        # of ~200 usable) so tiles are never recycled, but descriptors are
        # issued incrementally: a big upfront burst starves the sync queue
        # (which also relays cross-engine semaphores) for ~40us and drags
        # the whole pipeline into a slow limit cycle. Burst just enough to
        # cover the prologue, then 2 descriptors per iteration.
        PREFETCH = 4
        dma_load_xa(0)
        for d in range(1, min(PREFETCH, Gd)):
            dma_load_xt(d)
            dma_load_xa(d)

        for dd in range(Gd + 4):
            if dd < Gd:
                psum_h = ph_pool.tile([HH, 2 * GROUP], dt.float32)
                for n0 in range(0, 2 * GROUP, 512):
                    nc.tensor.matmul(
                        psum_h[:, n0 : n0 + 512],
                        lhsT=w1_sb[:],
                        rhs=xt_slice(dd, n0, n0 + 512),
                        start=True, stop=True,
                        perf_mode=mybir.MatmulPerfMode.DoubleRow,
                    )
                phs[dd] = psum_h
            if 1 <= dd <= Gd:
                # tanh runs one iteration AFTER its MLP: every cross-engine
                # hop in the MLP->tanh->score->exp->scat->scatter chain has
                # >=1 full iteration of slack, so a late producer never
                # drags the pipeline into a slow limit cycle. Emitted first
                # on the ACT queue so the ph PSUM buffer frees earliest
                # (ph bufs=2 gives the next MLP exactly this much slack).
                dth = dd - 1
                psum_h = phs.pop(dth)
                th = th_pool.tile([HH, 2 * GROUP], dt.bfloat16)
                nc.scalar.activation(
                    th[:], psum_h[:], mybir.ActivationFunctionType.Tanh,
                    bias=b1_sb[:, 0:1], scale=ACT_SCALE,
                )
                ths[dth] = th

            if 2 <= dd <= Gd + 1:
                d1 = dd - 2
                xtts.pop(d1, None)
                th = ths.pop(d1)
                psum_s = ps_pool.tile([P, 8], dt.float32)
                for si in range(8):
                    nc.tensor.matmul(
                        psum_s[:, si : si + 1],
                        lhsT=th[:, si * P : (si + 1) * P],
                        rhs=w2_sb[:],
                        start=True, stop=True,
                    )
                e8 = e_pool.tile([P, 8], dt.float32)
                nc.scalar.activation(
                    e8[:], psum_s[:],
                    mybir.ActivationFunctionType.Exp,
                    bias=b2_sb[:, 0:1], scale=1.0,
                )

            if 2 <= dd <= Gd + 1:
                d1 = dd - 2
                # scat build for the whole supergroup: 2 broadcast DVE ops
                sel = sel_pool.tile([P, 8, WIN], dt.bfloat16)
                nc.vector.tensor_tensor(
                    out=sel[:],
                    in0=iota_sb[:].unsqueeze(1).broadcast_to([P, 8, WIN]),
                    in1=bl_sb[:, 8 * d1 : 8 * d1 + 8]
                    .unsqueeze(2)
                    .broadcast_to([P, 8, WIN]),
                    op=mybir.AluOpType.is_equal,
                )
                scat = scat_pool.tile([P, 8, WIN], dt.bfloat16)
                nc.vector.tensor_tensor(
                    out=scat[:],
                    in0=sel[:],
                    in1=e8[:].unsqueeze(2).broadcast_to([P, 8, WIN]),
                    op=mybir.AluOpType.mult,
                )
                extras = {}
                for si in range(8):
                    t = d1 * 8 + si
                    for k in range(1, len(tile_plan[t])):
                        selx = ex_pool.tile([P, WIN], dt.bfloat16)
                        nc.vector.tensor_tensor(
                            out=selx[:],
                            in0=iota_sb[:],
                            in1=bl_sb[:, tile_plan[t][k][1] : tile_plan[t][k][1] + 1]
                            .broadcast_to([P, WIN]),
                            op=mybir.AluOpType.is_equal,
                        )
                        scx = ex_pool.tile([P, WIN], dt.bfloat16)
                        nc.vector.tensor_tensor(
                            out=scx[:],
                            in0=selx[:],
                            in1=e8[:, si : si + 1].broadcast_to([P, WIN]),
                            op=mybir.AluOpType.mult,
                        )
                        extras.setdefault(si, []).append((k, scx))
                scats[d1] = (scat, extras)

            if 4 <= dd:
                d2 = dd - 4
                scat, extras = scats.pop(d2)
                for q in (0, 1):
                    g = 2 * d2 + q
                    for s in range(4):
                        si = 4 * q + s
                        t = g * 4 + s
                        w, col, first, last = tile_plan[t][0]
                        nc.tensor.matmul(
                            acc[WIN * w : WIN * (w + 1), :],
                            lhsT=scat[:, si, :],
                            rhs=xa_slice(g, s),
                            start=first, stop=last,
                            skip_group_check=True,
                            tile_position=(0, WIN * w),
                        )
                        for (k, scx) in extras.get(si, ()):
                            w2_, col2, first2, last2 = tile_plan[t][k]
                            nc.tensor.matmul(
                                acc[WIN * w2_ : WIN * (w2_ + 1), :],
                                lhsT=scx[:],
                                rhs=xa_slice(g, s),
                                start=first2, stop=last2,
                                skip_group_check=True,
                                tile_position=(0, WIN * w2_),
                            )
                xtts.pop(d2, None)
                xats.pop(d2)

            for w_fin in fin_sched.get(dd, ()):
                finalize_w(w_fin)

            d_next = dd + PREFETCH
            if d_next < Gd:
                dma_load_xt(d_next)
                dma_load_xa(d_next)

        for it in sorted(fin_sched):
            if it >= Gd + 4:
                for w_fin in fin_sched[it]:
                    finalize_w(w_fin)

    nc.compile()
    return nc


def _maybe_enable_trace():
    """Dev-only NTFF profiling: register the axon NTFF hook if available.
    Inert when ATT_POOL_TRACE is unset (the grading path)."""
    if os.environ.get("ATT_POOL_TRACE") != "1":
        return False
    try:
        import sys
        import types

        import trn_agent_boot.trn_boot as tb

        hook = tb._ntff_profile_via_ctypes("/opt/axon/libaxon_pjrt.so")
        mod = types.ModuleType("antenv.axon_hooks")
        mod.get_axon_ntff_profile_hook = lambda: hook
        mod.set_axon_ntff_profile_hook = lambda h: None
        sys.modules["antenv.axon_hooks"] = mod

        import concourse.bass_utils as bu

        bu.upload_artifacts = lambda tmpdir: "local://" + str(tmpdir)
        return True
    except Exception as e:  # pragma: no cover - dev path only
        print("trace setup failed:", e)
        return False


def kernel(x, batch, W1, b1, W2, b2):
    from concourse.bass_utils import run_bass_kernel_spmd

    x = np.asarray(x, dtype=np.float32)
    batch = np.asarray(batch).astype(np.int64)
    W1 = np.asarray(W1, dtype=np.float32)
    b1 = np.asarray(b1, dtype=np.float32)
    W2 = np.asarray(W2, dtype=np.float32)
    b2 = np.asarray(b2, dtype=np.float32)

    bf16 = ml_dtypes.bfloat16
    f8 = ml_dtypes.float8_e4m3

    bounds = np.searchsorted(batch, np.arange(0, NUM_GRAPHS + 1, SBL))
    shard = np.diff(bounds)
    npad_half = int(-(-int(shard.max()) // GROUP) * GROUP)
    npad = 2 * npad_half
    Gd = npad // SUPER
    T = npad // P

    f8e3 = ml_dtypes.float8_e3m4
    x_8 = (x * XSCALE).astype(f8)
    x_a8 = np.clip(x * XA_SCALE, -15.5, 15.5).astype(f8e3)
    xct_h = np.zeros((N_CORES, Gd, P, 2, SUPER), dtype=f8)
    xca_h = np.zeros((N_CORES, Gd, P, 2 * XA_BLK), dtype=f8e3)

    # per-core local segment id per padded node slot (PAD = -1)
    lseg_h = np.full((N_CORES, npad), -1, dtype=np.int64)
    for c in range(N_CORES):
        xa = np.zeros((npad, XW), dtype=f8e3)
        xt = np.zeros((2, P, npad), dtype=f8)
        for hh in range(2):
            idx = 2 * c + hh
            s0, s1 = int(bounds[idx]), int(bounds[idx + 1])
            n = s1 - s0
            o = hh * npad_half
            xa[o : o + n, :HIDDEN] = x_a8[s0:s1]
            xa[o : o + n, HIDDEN] = f8e3(XA_SCALE)
            xt[0, :, o : o + n] = x_8[s0:s1, 0:P].T
            xt[1, :, o : o + n] = x_8[s0:s1, P:HIDDEN].T
            lseg_h[c, o : o + n] = batch[s0:s1] - c * BL
        xa_sw = (
            xa.reshape(Gd, 2, 4, P, XW)
            .transpose(0, 3, 1, 2, 4)
            .reshape(Gd, P, 2 * XA_BLK)
        )
        xca_h[c] = xa_sw
        xct_h[c] = xt.reshape(2, P, Gd, SUPER).transpose(2, 1, 0, 3)

    # Tile plan, shared by all 8 cores (one Bass program): per tile t a
    # list of (window, blh_col, first, last) matmul entries. Window = a
    # 32-aligned block of the 128 local segments (PSUM out partitions must
    # be 32-aligned). Each core's tile spans <=2 adjacent windows (every
    # graph here has >=128 nodes); the shared plan takes the UNION of
    # windows across cores per tile. A core contributes zeros to a window
    # its nodes don't belong to: bl = lseg - 32*w falls outside iota 0..31
    # so is_equal matches nothing.
    win_union = []
    for t in range(T):
        wins = set()
        for c in range(N_CORES):
            ls = lseg_h[c, t * P : (t + 1) * P]
            real = ls >= 0
            if not real.any():
                continue
            smin, smax = int(ls[real].min()), int(ls[real].max())
            assert smax // WIN - smin // WIN <= 1, "tile spans >2 windows"
            wins.update(range(smin // WIN, smax // WIN + 1))
        win_union.append(sorted(wins) or [0])

    plan_final = []
    ecol = 0
    for t in range(T):
        ent = []
        for k, w in enumerate(win_union[t]):
            col = t if k == 0 else T + ecol
            if k > 0:
                ecol += 1
            ent.append([w, col, False, False])
        plan_final.append(ent)
    n_extra = ecol
    # first/last touch per window in PE issue order (tile order, primary
    # then extra within a tile): first gets start=True (resets the psum
    # window - every window matmul writes all 32 rows, zeros included),
    # last gets stop=True.
    seen_first, last_touch = {}, {}
    for t in range(T):
        for k in range(len(plan_final[t])):
            w = plan_final[t][k][0]
            if w not in seen_first:
                seen_first[w] = (t, k)
            last_touch[w] = (t, k)
    for w, (t, k) in seen_first.items():
        plan_final[t][k][2] = True
    for w, (t, k) in last_touch.items():
        plan_final[t][k][3] = True
    plan_final = [[tuple(e) for e in ent] for ent in plan_final]

    blh_h = np.full((N_CORES, P, T + n_extra), 300.0, dtype=np.float32)
    for c in range(N_CORES):
        for t in range(T):
            ls = lseg_h[c, t * P : (t + 1) * P]
            real = ls >= 0
            if not real.any():
                continue
            for (w, col, _f, _l) in plan_final[t]:
                blh_h[c, :, col] = np.where(real, ls - WIN * w, 300).astype(
                    np.float32
                )

    w1_8 = np.ascontiguousarray((W1 * WSCALE).astype(f8).reshape(2, P, HH))
    w2_bf = np.ascontiguousarray(W2.astype(bf16).reshape(HH, 1))
    b1_col = np.ascontiguousarray(b1.reshape(HH, 1))
    b2_col = np.full((P, 1), float(b2[0]), dtype=np.float32)
    iota_h = np.ascontiguousarray(
        np.broadcast_to(np.arange(WIN, dtype=np.float32), (P, WIN))
    )

    trace = _maybe_enable_trace()
    nc = _build_bass(npad, plan_final, n_extra)

    in_maps = []
    for c in range(N_CORES):
        in_maps.append(
            {
                "xct": xct_h[c],
                "xca": xca_h[c],
                "blh": blh_h[c],
                "w1": w1_8,
                "w2": w2_bf,
                "b1": b1_col,
                "b2c": b2_col,
                "iotad": iota_h,
            }
        )

    res = run_bass_kernel_spmd(
        nc, in_maps, core_ids=list(range(N_CORES)), trace=trace
    )
    if trace and res.exec_time_ns is not None:
        print(f"HW exec time: {res.exec_time_ns} ns")
        if res.instructions_and_trace:
            print("trace:", res.instructions_and_trace[1])
        if res.profile_json:
            print("profile_json:", res.profile_json)

    out = np.concatenate([res.results[c]["out"] for c in range(N_CORES)], axis=0)
    assert out.shape == (NUM_GRAPHS, HIDDEN)
    return np.ascontiguousarray(out.astype(np.float32))
